# revision 1
# baseline (speedup 1.0000x reference)
"""TRN2 Bass kernel for nn_ChEst: Elman-RNN channel estimation scan.

  est[t] = tanh(x[t] @ W_ih.T + b_ih + est[t-1] @ W_hh.T + b_hh),  est[-1] = 0
  x: [16384, 512] fp32 -> est: [16384, 512] fp32

Strategy (8 NeuronCores, no collectives):
  The recurrence Jacobian diag(1-h^2) @ W_hh.T is strongly contractive
  (~0.46x per step for this weight scale), so a scan started from h=0 at
  (t0 - K) converges to the exact trajectory to <2e-7 within ~24 steps.
  We split the 16384 rows into 2048 chunks of L=8 and give each core
  B=256 chunks (a contiguous 2048-row block).  Each core runs its B
  chunks in lockstep: one step is a [512,512] x [512,256] matmul + tanh
  on the tensor/scalar engines, with a K=32-step warmup whose inputs come
  from the halo rows before the block (core 0 pads with A=0, which keeps
  h exactly 0 through warmup).  Per core:
    phase A: DMA x block, transpose to X^T on the PE (fp32)
    phase B: A^T = W_ih @ X^T + (b_ih+b_hh) (float32r matmuls, fused bias)
    phase C: 40 steps of H^T <- tanh(A_step + W_hh @ H^T) (float32r)
    phase D: transpose H^T back to natural rows, DMA out
  float32r is fp32 storage with PE operand rounding (~13 mantissa bits);
  measured end-to-end relative error ~2e-4 (the contraction keeps the
  per-step rounding from accumulating).
"""

from contextlib import ExitStack

import numpy as np

import concourse.tile as tile
from concourse import bacc, mybir
from concourse.masks import make_identity
from concourse.bass_utils import run_bass_kernel_spmd

F32 = mybir.dt.float32
F32R = mybir.dt.float32r
COL = 512
SEQ = 16384
NCORES = 8
CT = COL // 128  # 4 column tiles
B = 256          # chunks per core (batch width of the lockstep scan)
K = 32           # warmup steps


def _build(reps=1):
    rows = SEQ // NCORES           # 2048 output rows per core
    L = rows // B                  # chunk length (8)
    steps = L + K                  # 40
    xrows = rows + K               # input rows incl. halo
    xpad = ((xrows + 127) // 128) * 128

    nc = bacc.Bacc("TRN2", target_bir_lowering=False, debug=False,
                   num_devices=NCORES)
    x_in = nc.dram_tensor("x_blk", [xpad, COL], F32, kind="ExternalInput").ap()
    wih_in = nc.dram_tensor("w_ihT", [COL, COL], F32, kind="ExternalInput").ap()
    whh_in = nc.dram_tensor("w_hhT", [COL, COL], F32, kind="ExternalInput").ap()
    bias_in = nc.dram_tensor("bias", [128, CT], F32, kind="ExternalInput").ap()
    amask_in = nc.dram_tensor("amask", [128, K], F32, kind="ExternalInput").ap()
    out_dram = nc.dram_tensor("out_blk", [rows, COL], F32, kind="ExternalOutput").ap()

    with tile.TileContext(nc) as tc, ExitStack() as ctx:
        const = ctx.enter_context(tc.tile_pool(name="const", bufs=1))
        xnp = ctx.enter_context(tc.tile_pool(name="xn", bufs=4))
        big = ctx.enter_context(tc.tile_pool(name="big", bufs=1))
        hts = ctx.enter_context(tc.tile_pool(name="hts", bufs=1))
        onp = ctx.enter_context(tc.tile_pool(name="on", bufs=3))
        psp = ctx.enter_context(tc.tile_pool(name="ps", bufs=8, space="PSUM"))

        # constants
        ident = const.tile([128, 128], F32, name="ident", tag="ident")
        make_identity(nc, ident[:])
        wih = [const.tile([128, COL], F32, name=f"wih{c}", tag=f"wih{c}") for c in range(CT)]
        whh = [const.tile([128, COL], F32, name=f"whh{c}", tag=f"whh{c}") for c in range(CT)]
        for c in range(CT):
            nc.sync.dma_start(wih[c][:], wih_in[128 * c:128 * (c + 1), :])
            nc.sync.dma_start(whh[c][:], whh_in[128 * c:128 * (c + 1), :])
        # float32r copies (PE requires producers to round operands to f32r)
        wihr = [const.tile([128, COL], F32R, name=f"wihr{c}", tag=f"wihr{c}") for c in range(CT)]
        whhr = [const.tile([128, COL], F32R, name=f"whhr{c}", tag=f"whhr{c}") for c in range(CT)]
        for c in range(CT):
            nc.vector.tensor_copy(wihr[c][:], wih[c][:])
            nc.vector.tensor_copy(whhr[c][:], whh[c][:])
        bias = const.tile([128, CT], F32, name="bias", tag="bias")
        nc.sync.dma_start(bias[:], bias_in[:])
        amask = const.tile([128, K], F32, name="amask", tag="amask")
        nc.sync.dma_start(amask[:], amask_in[:])

        rep_ctx = tc.For_i(0, reps, 1) if reps > 1 else None
        if rep_ctx is not None:
            rep_ctx.__enter__()

        # --- phase A: load x, build X^T (f32r) via PE transposes ---
        xt = [big.tile([128, xpad], F32R, name=f"xt{c}", tag=f"xt{c}") for c in range(CT)]
        for rt in range(xpad // 128):
            xn = xnp.tile([128, COL], F32, name="xn", tag="xn")
            nc.sync.dma_start(xn[:], x_in[128 * rt:128 * (rt + 1), :])
            for c in range(CT):
                ps = psp.tile([128, 128], F32, name="ps", tag="ps", bufs=4)
                nc.tensor.transpose(ps[:], xn[:, 128 * c:128 * (c + 1)], ident[:])
                nc.vector.tensor_copy(xt[c][:, 128 * rt:128 * (rt + 1)], ps[:])

        # --- phase B: A^T = W_ih @ X^T + bias ---
        # at_all[:, o*xrows + n] = A^T[128o:128(o+1), n];  n = b*L + s
        at_all = big.tile([128, CT * xrows], F32, name="at_all", tag="at_all")
        nchunks = [(n0, min(512, xrows - n0)) for n0 in range(0, xrows, 512)]
        for o in range(CT):
            for (n0, nl) in nchunks:
                ps = psp.tile([128, 512], F32, name="ps", tag="ps", bufs=4)
                for c in range(CT):
                    nc.tensor.matmul(ps[:, :nl], wihr[c][:, 128 * o:128 * (o + 1)],
                                     xt[c][:, n0:n0 + nl],
                                     start=(c == 0), stop=(c == CT - 1))
                nc.scalar.activation(at_all[:, o * xrows + n0:o * xrows + n0 + nl],
                                     ps[:, :nl],
                                     mybir.ActivationFunctionType.Identity,
                                     bias=bias[:, o:o + 1])
        # zero the A warmup region on core 0 (amask = 0 there, 1 elsewhere)
        for o in range(CT):
            nc.vector.tensor_mul(at_all[:, o * xrows:o * xrows + K],
                                 at_all[:, o * xrows:o * xrows + K], amask[:])
        at3 = at_all.rearrange("p (o n) -> p o n", o=CT)

        # --- phase C: the lockstep recurrence ---
        ht = [[hts.tile([128, B], F32R, name=f"ht{p}{c}", tag=f"ht{p}{c}")
               for c in range(CT)] for p in range(2)]
        hz = hts.tile([128, B], F32, name="hz", tag="hz")
        nc.vector.memset(hz[:], 0.0)
        for c in range(CT):
            nc.vector.tensor_copy(ht[0][c][:], hz[:])
        htout = [big.tile([128, rows], F32, name=f"ho{o}", tag=f"ho{o}")
                 for o in range(CT)]

        for s in range(steps):
            cur, nxt = s % 2, (s + 1) % 2
            for o in range(CT):
                ps = psp.tile([128, B], F32, name="psc", tag="psc", bufs=2)
                for c in range(CT):
                    nc.tensor.matmul(ps[:], whhr[c][:, 128 * o:128 * (o + 1)],
                                     ht[cur][c][:],
                                     start=(c == 0), stop=(c == CT - 1))
                a_sl = at3[:, o, s:s + (B - 1) * L + 1:L]
                nc.vector.tensor_add(ht[nxt][o][:], ps[:], a_sl)
                nc.scalar.activation(ht[nxt][o][:], ht[nxt][o][:],
                                     mybir.ActivationFunctionType.Tanh)
                if s >= K:
                    m = s - K
                    nc.vector.tensor_copy(htout[o][:, m:m + (B - 1) * L + 1:L],
                                          ht[nxt][o][:])

        # --- phase D: transpose back to natural rows, DMA out ---
        for rt in range(rows // 128):
            on = onp.tile([128, COL], F32, name="on", tag="on")
            for o in range(CT):
                ps = psp.tile([128, 128], F32, name="ps", tag="ps", bufs=4)
                nc.tensor.transpose(ps[:], htout[o][:, 128 * rt:128 * (rt + 1)],
                                    ident[:])
                nc.vector.tensor_copy(on[:, 128 * o:128 * (o + 1)], ps[:])
            nc.sync.dma_start(out_dram[128 * rt:128 * (rt + 1), :], on[:])

        if rep_ctx is not None:
            rep_ctx.__exit__(None, None, None)

    nc.compile()
    meta = dict(B=B, K=K, L=rows // B, steps=rows // B + K, rows=rows,
                xrows=xrows, xpad=xpad)
    return nc, meta


def _host_inputs(x, W_ih, W_hh, b_ih, b_hh, meta):
    rows, xrows, xpad, Kw = meta["rows"], meta["xrows"], meta["xpad"], meta["K"]
    x = np.ascontiguousarray(x, dtype=np.float32)
    w_ihT = np.ascontiguousarray(np.asarray(W_ih, np.float32).T)
    w_hhT = np.ascontiguousarray(np.asarray(W_hh, np.float32).T)
    bv = np.asarray(b_ih, np.float32) + np.asarray(b_hh, np.float32)
    bias = np.ascontiguousarray(bv.reshape(CT, 128).T, dtype=np.float32)
    in_maps = []
    for k in range(NCORES):
        xb = np.zeros((xpad, COL), np.float32)
        lo = k * rows - Kw
        off = max(0, -lo)
        xb[off:xrows] = x[lo + off:lo + xrows]
        amask = (np.zeros if k == 0 else np.ones)((128, Kw), np.float32)
        in_maps.append({"x_blk": xb, "w_ihT": w_ihT, "w_hhT": w_hhT,
                        "bias": bias, "amask": amask})
    return in_maps


_CACHE = {}


def kernel(x, W_ih, W_hh, b_ih, b_hh):
    if "nc" not in _CACHE:
        _CACHE["nc"], _CACHE["meta"] = _build()
    nc, meta = _CACHE["nc"], _CACHE["meta"]
    in_maps = _host_inputs(x, W_ih, W_hh, b_ih, b_hh, meta)
    res = run_bass_kernel_spmd(nc, in_maps, list(range(NCORES)))
    out = np.concatenate([res.results[k]["out_blk"] for k in range(NCORES)], axis=0)
    return out.astype(np.float32)


# revision 2
# speedup vs baseline: 1.5443x; 1.5443x over previous
"""TRN2 Bass kernel for nn_ChEst: Elman-RNN channel estimation scan.

  est[t] = tanh(x[t] @ W_ih.T + b_ih + est[t-1] @ W_hh.T + b_hh),  est[-1] = 0
  x: [16384, 512] fp32 -> est: [16384, 512] fp32

Strategy (8 NeuronCores, no collectives):
  The recurrence Jacobian diag(1-h^2) @ W_hh.T is strongly contractive
  (~0.46x per step for this weight scale), so a scan started from h=0 at
  (t0 - K) converges to the exact trajectory to <2e-7 within ~24 steps.
  We split the 16384 rows into 2048 chunks of L=8 and give each core
  B=256 chunks (a contiguous 2048-row block).  Each core runs its B
  chunks in lockstep: one step is a [512,512] x [512,256] matmul + tanh
  on the tensor/scalar engines, with a K=16-step warmup whose inputs come
  from the halo rows before the block (core 0 pads with A=0, which keeps
  h exactly 0 through warmup).  Per core:
    phase A: DMA x block, transpose to X^T on the PE (fp32)
    phase B: A^T = W_ih @ X^T + (b_ih+b_hh) (float32r matmuls, fused bias)
    phase C: 24 steps of H^T <- tanh(A_step + W_hh @ H^T) (float32r)
    phase D: transpose H^T back to natural rows, DMA out
  float32r is fp32 storage with PE operand rounding (~13 mantissa bits);
  measured end-to-end relative error ~2e-4 (the contraction keeps the
  per-step rounding from accumulating).
"""

from contextlib import ExitStack

import numpy as np

import concourse.tile as tile
from concourse import bacc, mybir
from concourse.masks import make_identity
from concourse.bass_utils import run_bass_kernel_spmd

F32 = mybir.dt.float32
F32R = mybir.dt.float32r
COL = 512
SEQ = 16384
NCORES = 8
CT = COL // 128  # 4 column tiles
B = 256          # chunks per core (batch width of the lockstep scan)
K = 16           # warmup steps (converges to 3e-6, well under the f32r noise floor)


def _build(reps=1):
    rows = SEQ // NCORES           # 2048 output rows per core
    L = rows // B                  # chunk length (8)
    steps = L + K                  # 40
    xrows = rows + K               # input rows incl. halo
    xpad = ((xrows + 127) // 128) * 128

    nc = bacc.Bacc("TRN2", target_bir_lowering=False, debug=False,
                   num_devices=NCORES)
    x_in = nc.dram_tensor("x_blk", [xpad, COL], F32, kind="ExternalInput").ap()
    wih_in = nc.dram_tensor("w_ihT", [COL, COL], F32, kind="ExternalInput").ap()
    whh_in = nc.dram_tensor("w_hhT", [COL, COL], F32, kind="ExternalInput").ap()
    bias_in = nc.dram_tensor("bias", [128, CT], F32, kind="ExternalInput").ap()
    amask_in = nc.dram_tensor("amask", [128, K], F32, kind="ExternalInput").ap()
    out_dram = nc.dram_tensor("out_blk", [rows, COL], F32, kind="ExternalOutput").ap()

    with tile.TileContext(nc) as tc, ExitStack() as ctx:
        const = ctx.enter_context(tc.tile_pool(name="const", bufs=1))
        xnp = ctx.enter_context(tc.tile_pool(name="xn", bufs=4))
        big = ctx.enter_context(tc.tile_pool(name="big", bufs=1))
        hts = ctx.enter_context(tc.tile_pool(name="hts", bufs=1))
        onp = ctx.enter_context(tc.tile_pool(name="on", bufs=3))
        psp = ctx.enter_context(tc.tile_pool(name="ps", bufs=8, space="PSUM"))

        # constants
        ident = const.tile([128, 128], F32, name="ident", tag="ident")
        make_identity(nc, ident[:])
        wih = [const.tile([128, COL], F32, name=f"wih{c}", tag=f"wih{c}") for c in range(CT)]
        whh = [const.tile([128, COL], F32, name=f"whh{c}", tag=f"whh{c}") for c in range(CT)]
        for c in range(CT):
            nc.sync.dma_start(wih[c][:], wih_in[128 * c:128 * (c + 1), :])
            nc.sync.dma_start(whh[c][:], whh_in[128 * c:128 * (c + 1), :])
        # float32r copies (PE requires producers to round operands to f32r)
        wihr = [const.tile([128, COL], F32R, name=f"wihr{c}", tag=f"wihr{c}") for c in range(CT)]
        whhr = [const.tile([128, COL], F32R, name=f"whhr{c}", tag=f"whhr{c}") for c in range(CT)]
        for c in range(CT):
            nc.vector.tensor_copy(wihr[c][:], wih[c][:])
            nc.vector.tensor_copy(whhr[c][:], whh[c][:])
        bias = const.tile([128, CT], F32, name="bias", tag="bias")
        nc.sync.dma_start(bias[:], bias_in[:])
        amask = const.tile([128, K], F32, name="amask", tag="amask")
        nc.sync.dma_start(amask[:], amask_in[:])

        rep_ctx = tc.For_i(0, reps, 1) if reps > 1 else None
        if rep_ctx is not None:
            rep_ctx.__enter__()

        # --- phase A: load x, build X^T (f32r) via PE transposes ---
        xt = [big.tile([128, xpad], F32R, name=f"xt{c}", tag=f"xt{c}") for c in range(CT)]
        for rt in range(xpad // 128):
            xn = xnp.tile([128, COL], F32, name="xn", tag="xn")
            nc.sync.dma_start(xn[:], x_in[128 * rt:128 * (rt + 1), :])
            for c in range(CT):
                ps = psp.tile([128, 128], F32, name="ps", tag="ps", bufs=4)
                nc.tensor.transpose(ps[:], xn[:, 128 * c:128 * (c + 1)], ident[:])
                nc.vector.tensor_copy(xt[c][:, 128 * rt:128 * (rt + 1)], ps[:])

        # --- phase B: A^T = W_ih @ X^T + bias ---
        # at_all[:, o*xrows + n] = A^T[128o:128(o+1), n];  n = b*L + s
        at_all = big.tile([128, CT * xrows], F32, name="at_all", tag="at_all")
        nchunks = [(n0, min(512, xrows - n0)) for n0 in range(0, xrows, 512)]
        for o in range(CT):
            for (n0, nl) in nchunks:
                ps = psp.tile([128, 512], F32, name="ps", tag="ps", bufs=4)
                for c in range(CT):
                    nc.tensor.matmul(ps[:, :nl], wihr[c][:, 128 * o:128 * (o + 1)],
                                     xt[c][:, n0:n0 + nl],
                                     start=(c == 0), stop=(c == CT - 1))
                nc.scalar.activation(at_all[:, o * xrows + n0:o * xrows + n0 + nl],
                                     ps[:, :nl],
                                     mybir.ActivationFunctionType.Identity,
                                     bias=bias[:, o:o + 1])
        # zero the A warmup region on core 0 (amask = 0 there, 1 elsewhere)
        for o in range(CT):
            nc.vector.tensor_mul(at_all[:, o * xrows:o * xrows + K],
                                 at_all[:, o * xrows:o * xrows + K], amask[:])
        at3 = at_all.rearrange("p (o n) -> p o n", o=CT)

        # --- phase C: the lockstep recurrence ---
        ht = [[hts.tile([128, B], F32R, name=f"ht{p}{c}", tag=f"ht{p}{c}")
               for c in range(CT)] for p in range(2)]
        hz = hts.tile([128, B], F32, name="hz", tag="hz")
        nc.vector.memset(hz[:], 0.0)
        for c in range(CT):
            nc.vector.tensor_copy(ht[0][c][:], hz[:])
        htout = [big.tile([128, rows], F32, name=f"ho{o}", tag=f"ho{o}")
                 for o in range(CT)]

        for s in range(steps):
            cur, nxt = s % 2, (s + 1) % 2
            for o in range(CT):
                ps = psp.tile([128, B], F32, name="psc", tag="psc", bufs=2)
                for c in range(CT):
                    nc.tensor.matmul(ps[:], whhr[c][:, 128 * o:128 * (o + 1)],
                                     ht[cur][c][:],
                                     start=(c == 0), stop=(c == CT - 1))
                a_sl = at3[:, o, s:s + (B - 1) * L + 1:L]
                nc.vector.tensor_add(ht[nxt][o][:], ps[:], a_sl)
                nc.scalar.activation(ht[nxt][o][:], ht[nxt][o][:],
                                     mybir.ActivationFunctionType.Tanh)
                if s >= K:
                    m = s - K
                    nc.vector.tensor_copy(htout[o][:, m:m + (B - 1) * L + 1:L],
                                          ht[nxt][o][:])

        # --- phase D: transpose back to natural rows, DMA out ---
        for rt in range(rows // 128):
            on = onp.tile([128, COL], F32, name="on", tag="on")
            for o in range(CT):
                ps = psp.tile([128, 128], F32, name="ps", tag="ps", bufs=4)
                nc.tensor.transpose(ps[:], htout[o][:, 128 * rt:128 * (rt + 1)],
                                    ident[:])
                nc.vector.tensor_copy(on[:, 128 * o:128 * (o + 1)], ps[:])
            nc.sync.dma_start(out_dram[128 * rt:128 * (rt + 1), :], on[:])

        if rep_ctx is not None:
            rep_ctx.__exit__(None, None, None)

    nc.compile()
    meta = dict(B=B, K=K, L=rows // B, steps=rows // B + K, rows=rows,
                xrows=xrows, xpad=xpad)
    return nc, meta


def _host_inputs(x, W_ih, W_hh, b_ih, b_hh, meta):
    rows, xrows, xpad, Kw = meta["rows"], meta["xrows"], meta["xpad"], meta["K"]
    x = np.ascontiguousarray(x, dtype=np.float32)
    w_ihT = np.ascontiguousarray(np.asarray(W_ih, np.float32).T)
    w_hhT = np.ascontiguousarray(np.asarray(W_hh, np.float32).T)
    bv = np.asarray(b_ih, np.float32) + np.asarray(b_hh, np.float32)
    bias = np.ascontiguousarray(bv.reshape(CT, 128).T, dtype=np.float32)
    in_maps = []
    for k in range(NCORES):
        xb = np.zeros((xpad, COL), np.float32)
        lo = k * rows - Kw
        off = max(0, -lo)
        xb[off:xrows] = x[lo + off:lo + xrows]
        amask = (np.zeros if k == 0 else np.ones)((128, Kw), np.float32)
        in_maps.append({"x_blk": xb, "w_ihT": w_ihT, "w_hhT": w_hhT,
                        "bias": bias, "amask": amask})
    return in_maps


_CACHE = {}


def kernel(x, W_ih, W_hh, b_ih, b_hh):
    if "nc" not in _CACHE:
        _CACHE["nc"], _CACHE["meta"] = _build()
    nc, meta = _CACHE["nc"], _CACHE["meta"]
    in_maps = _host_inputs(x, W_ih, W_hh, b_ih, b_hh, meta)
    res = run_bass_kernel_spmd(nc, in_maps, list(range(NCORES)))
    out = np.concatenate([res.results[k]["out_blk"] for k in range(NCORES)], axis=0)
    return out.astype(np.float32)


# revision 3
# speedup vs baseline: 1.5869x; 1.0276x over previous
"""TRN2 Bass kernel for nn_ChEst: Elman-RNN channel estimation scan.

  est[t] = tanh(x[t] @ W_ih.T + b_ih + est[t-1] @ W_hh.T + b_hh),  est[-1] = 0
  x: [16384, 512] fp32 -> est: [16384, 512] fp32

Strategy (8 NeuronCores, no collectives):
  The recurrence Jacobian diag(1-h^2) @ W_hh.T is strongly contractive
  (~0.46x per step for this weight scale), so a scan started from h=0 at
  (t0 - K) converges to the exact trajectory to <2e-7 within ~24 steps.
  We split the 16384 rows into 2048 chunks of L=8 and give each core
  B=256 chunks (a contiguous 2048-row block).  Each core runs its B
  chunks in lockstep: one step is a [512,512] x [512,256] matmul + tanh
  on the tensor/scalar engines, with a K=16-step warmup whose inputs come
  from the halo rows before the block (core 0 pads with A=0, which keeps
  h exactly 0 through warmup).  Per core:
    phase A: DMA x block, transpose to X^T on the PE (fp32)
    phase B: A^T = W_ih @ X^T + (b_ih+b_hh) (float32r matmuls, fused bias)
    phase C: 24 steps of H^T <- tanh(A_step + W_hh @ H^T) (float32r)
    phase D: transpose H^T back to natural rows, DMA out
  float32r is fp32 storage with PE operand rounding (~13 mantissa bits);
  measured end-to-end relative error ~2e-4 (the contraction keeps the
  per-step rounding from accumulating).
"""

from contextlib import ExitStack

import numpy as np

import concourse.tile as tile
from concourse import bacc, mybir
from concourse.masks import make_identity
from concourse.bass_utils import run_bass_kernel_spmd

F32 = mybir.dt.float32
F32R = mybir.dt.float32r
COL = 512
SEQ = 16384
NCORES = 8
CT = COL // 128  # 4 column tiles
B = 256          # chunks per core (batch width of the lockstep scan)
K = 16           # warmup steps (converges to 3e-6, well under the f32r noise floor)


def _build(reps=1):
    rows = SEQ // NCORES           # 2048 output rows per core
    L = rows // B                  # chunk length (8)
    steps = L + K                  # 40
    xrows = rows + K               # input rows incl. halo
    xpad = ((xrows + 127) // 128) * 128

    nc = bacc.Bacc("TRN2", target_bir_lowering=False, debug=False,
                   num_devices=NCORES)
    x_in = nc.dram_tensor("x_blk", [xpad, COL], F32, kind="ExternalInput").ap()
    wih_in = nc.dram_tensor("w_ihT", [COL, COL], F32, kind="ExternalInput").ap()
    whh_in = nc.dram_tensor("w_hhT", [COL, COL], F32, kind="ExternalInput").ap()
    bias_in = nc.dram_tensor("bias", [128, CT], F32, kind="ExternalInput").ap()
    amask_in = nc.dram_tensor("amask", [128, K], F32, kind="ExternalInput").ap()
    out_dram = nc.dram_tensor("out_blk", [rows, COL], F32, kind="ExternalOutput").ap()

    with tile.TileContext(nc) as tc, ExitStack() as ctx:
        const = ctx.enter_context(tc.tile_pool(name="const", bufs=1))
        xnp = ctx.enter_context(tc.tile_pool(name="xn", bufs=4))
        big = ctx.enter_context(tc.tile_pool(name="big", bufs=1))
        hts = ctx.enter_context(tc.tile_pool(name="hts", bufs=1))
        onp = ctx.enter_context(tc.tile_pool(name="on", bufs=3))
        psp = ctx.enter_context(tc.tile_pool(name="ps", bufs=8, space="PSUM"))

        # constants
        ident = const.tile([128, 128], F32, name="ident", tag="ident")
        make_identity(nc, ident[:])
        wih = [const.tile([128, COL], F32, name=f"wih{c}", tag=f"wih{c}") for c in range(CT)]
        whh = [const.tile([128, COL], F32, name=f"whh{c}", tag=f"whh{c}") for c in range(CT)]
        for c in range(CT):
            nc.sync.dma_start(wih[c][:], wih_in[128 * c:128 * (c + 1), :])
            nc.sync.dma_start(whh[c][:], whh_in[128 * c:128 * (c + 1), :])
        # float32r copies (PE requires producers to round operands to f32r)
        wihr = [const.tile([128, COL], F32R, name=f"wihr{c}", tag=f"wihr{c}") for c in range(CT)]
        whhr = [const.tile([128, COL], F32R, name=f"whhr{c}", tag=f"whhr{c}") for c in range(CT)]
        for c in range(CT):
            nc.vector.tensor_copy(wihr[c][:], wih[c][:])
            nc.vector.tensor_copy(whhr[c][:], whh[c][:])
        bias = const.tile([128, CT], F32, name="bias", tag="bias")
        nc.sync.dma_start(bias[:], bias_in[:])
        amask = const.tile([128, K], F32, name="amask", tag="amask")
        nc.sync.dma_start(amask[:], amask_in[:])

        rep_ctx = tc.For_i(0, reps, 1) if reps > 1 else None
        if rep_ctx is not None:
            rep_ctx.__enter__()

        # --- phase A: load x, build X^T (f32r) via PE transposes ---
        xt = [big.tile([128, xpad], F32R, name=f"xt{c}", tag=f"xt{c}") for c in range(CT)]
        for rt in range(xpad // 128):
            xn = xnp.tile([128, COL], F32, name="xn", tag="xn")
            nc.sync.dma_start(xn[:], x_in[128 * rt:128 * (rt + 1), :])
            for c in range(CT):
                ps = psp.tile([128, 128], F32, name="ps", tag="ps", bufs=4)
                nc.tensor.transpose(ps[:], xn[:, 128 * c:128 * (c + 1)], ident[:])
                nc.vector.tensor_copy(xt[c][:, 128 * rt:128 * (rt + 1)], ps[:])

        # --- phase B: A^T = W_ih @ X^T + bias ---
        # at_all[:, o*xrows + n] = A^T[128o:128(o+1), n];  n = b*L + s
        at_all = big.tile([128, CT * xrows], F32, name="at_all", tag="at_all")
        nchunks = [(n0, min(512, xrows - n0)) for n0 in range(0, xrows, 512)]
        for o in range(CT):
            for (n0, nl) in nchunks:
                ps = psp.tile([128, 512], F32, name="ps", tag="ps", bufs=4)
                for c in range(CT):
                    nc.tensor.matmul(ps[:, :nl], wihr[c][:, 128 * o:128 * (o + 1)],
                                     xt[c][:, n0:n0 + nl],
                                     start=(c == 0), stop=(c == CT - 1))
                nc.scalar.activation(at_all[:, o * xrows + n0:o * xrows + n0 + nl],
                                     ps[:, :nl],
                                     mybir.ActivationFunctionType.Identity,
                                     bias=bias[:, o:o + 1])
        # zero the A warmup region on core 0 (amask = 0 there, 1 elsewhere)
        for o in range(CT):
            nc.vector.tensor_mul(at_all[:, o * xrows:o * xrows + K],
                                 at_all[:, o * xrows:o * xrows + K], amask[:])
        at3 = at_all.rearrange("p (o n) -> p o n", o=CT)

        # --- phase C: the lockstep recurrence ---
        ht = [[hts.tile([128, B], F32R, name=f"ht{p}{c}", tag=f"ht{p}{c}")
               for c in range(CT)] for p in range(2)]
        hz = hts.tile([128, B], F32, name="hz", tag="hz")
        nc.vector.memset(hz[:], 0.0)
        for c in range(CT):
            nc.vector.tensor_copy(ht[0][c][:], hz[:])
        htout = [big.tile([128, rows], F32, name=f"ho{o}", tag=f"ho{o}")
                 for o in range(CT)]

        for s in range(steps):
            cur, nxt = s % 2, (s + 1) % 2
            for o in range(CT):
                ps = psp.tile([128, B], F32, name="psc", tag="psc", bufs=4)
                for c in range(CT):
                    nc.tensor.matmul(ps[:], whhr[c][:, 128 * o:128 * (o + 1)],
                                     ht[cur][c][:],
                                     start=(c == 0), stop=(c == CT - 1))
                a_sl = at3[:, o, s:s + (B - 1) * L + 1:L]
                nc.vector.tensor_add(ht[nxt][o][:], ps[:], a_sl)
                nc.scalar.activation(ht[nxt][o][:], ht[nxt][o][:],
                                     mybir.ActivationFunctionType.Tanh)
                if s >= K:
                    m = s - K
                    nc.vector.tensor_copy(htout[o][:, m:m + (B - 1) * L + 1:L],
                                          ht[nxt][o][:])

        # --- phase D: transpose back to natural rows, DMA out ---
        for rt in range(rows // 128):
            on = onp.tile([128, COL], F32, name="on", tag="on")
            for o in range(CT):
                ps = psp.tile([128, 128], F32, name="ps", tag="ps", bufs=4)
                nc.tensor.transpose(ps[:], htout[o][:, 128 * rt:128 * (rt + 1)],
                                    ident[:])
                nc.vector.tensor_copy(on[:, 128 * o:128 * (o + 1)], ps[:])
            nc.sync.dma_start(out_dram[128 * rt:128 * (rt + 1), :], on[:])

        if rep_ctx is not None:
            rep_ctx.__exit__(None, None, None)

    nc.compile()
    meta = dict(B=B, K=K, L=rows // B, steps=rows // B + K, rows=rows,
                xrows=xrows, xpad=xpad)
    return nc, meta


def _host_inputs(x, W_ih, W_hh, b_ih, b_hh, meta):
    rows, xrows, xpad, Kw = meta["rows"], meta["xrows"], meta["xpad"], meta["K"]
    x = np.ascontiguousarray(x, dtype=np.float32)
    w_ihT = np.ascontiguousarray(np.asarray(W_ih, np.float32).T)
    w_hhT = np.ascontiguousarray(np.asarray(W_hh, np.float32).T)
    bv = np.asarray(b_ih, np.float32) + np.asarray(b_hh, np.float32)
    bias = np.ascontiguousarray(bv.reshape(CT, 128).T, dtype=np.float32)
    in_maps = []
    for k in range(NCORES):
        xb = np.zeros((xpad, COL), np.float32)
        lo = k * rows - Kw
        off = max(0, -lo)
        xb[off:xrows] = x[lo + off:lo + xrows]
        amask = (np.zeros if k == 0 else np.ones)((128, Kw), np.float32)
        in_maps.append({"x_blk": xb, "w_ihT": w_ihT, "w_hhT": w_hhT,
                        "bias": bias, "amask": amask})
    return in_maps


_CACHE = {}


def kernel(x, W_ih, W_hh, b_ih, b_hh):
    if "nc" not in _CACHE:
        _CACHE["nc"], _CACHE["meta"] = _build()
    nc, meta = _CACHE["nc"], _CACHE["meta"]
    in_maps = _host_inputs(x, W_ih, W_hh, b_ih, b_hh, meta)
    res = run_bass_kernel_spmd(nc, in_maps, list(range(NCORES)))
    out = np.concatenate([res.results[k]["out_blk"] for k in range(NCORES)], axis=0)
    return out.astype(np.float32)


# revision 4
# speedup vs baseline: 1.9108x; 1.2041x over previous
"""TRN2 Bass kernel for nn_ChEst: Elman-RNN channel estimation scan.

  est[t] = tanh(x[t] @ W_ih.T + b_ih + est[t-1] @ W_hh.T + b_hh),  est[-1] = 0
  x: [16384, 512] fp32 -> est: [16384, 512] fp32

Strategy (8 NeuronCores, no collectives):
  The recurrence Jacobian diag(1-h^2) @ W_hh.T is strongly contractive
  (~0.46x per step for this weight scale), so a scan started from h=0 at
  (t0 - K) converges to the exact trajectory to <2e-7 within ~24 steps.
  We split the 16384 rows into 2048 chunks of L=8 and give each core
  B=256 chunks (a contiguous 2048-row block).  Each core runs its B
  chunks in lockstep: one step is a [512,512] x [512,256] matmul + tanh
  on the tensor/scalar engines, with a K=16-step warmup whose inputs come
  from the halo rows before the block (core 0 pads with A=0, which keeps
  h exactly 0 through warmup).  Per core:
    phase A: DMA x block, transpose to X^T on the PE (fp32)
    phase B: A^T = W_ih @ X^T + (b_ih+b_hh) (float32r matmuls, fused bias)
    phase C: 24 steps of H^T <- tanh(A_step + W_hh @ H^T) (float32r)
    phase D: transpose H^T back to natural rows, DMA out
  float32r is fp32 storage with PE operand rounding (~13 mantissa bits);
  measured end-to-end relative error ~2e-4 (the contraction keeps the
  per-step rounding from accumulating).
"""

from contextlib import ExitStack

import numpy as np

import concourse.tile as tile
from concourse import bacc, mybir
from concourse.masks import make_identity
from concourse.bass_utils import run_bass_kernel_spmd

F32 = mybir.dt.float32
F32R = mybir.dt.float32r
COL = 512
SEQ = 16384
NCORES = 8
CT = COL // 128  # 4 column tiles
B = 256          # chunks per core (batch width of the lockstep scan)
K = 12           # warmup steps (worst-case convergence 1e-4, under the f32r noise floor)


def _build(reps=1):
    rows = SEQ // NCORES           # 2048 output rows per core
    L = rows // B                  # chunk length (8)
    steps = L + K                  # 40
    xrows = rows + K               # input rows incl. halo
    xpad = ((xrows + 127) // 128) * 128

    nc = bacc.Bacc("TRN2", target_bir_lowering=False, debug=False,
                   num_devices=NCORES)
    x_in = nc.dram_tensor("x_blk", [xpad, COL], F32, kind="ExternalInput").ap()
    wih_in = nc.dram_tensor("w_ihT", [COL, COL], F32, kind="ExternalInput").ap()
    whh_in = nc.dram_tensor("w_hhT", [COL, COL], F32, kind="ExternalInput").ap()
    bias_in = nc.dram_tensor("bias", [128, CT], F32, kind="ExternalInput").ap()
    amask_in = nc.dram_tensor("amask", [128, K], F32, kind="ExternalInput").ap()
    out_dram = nc.dram_tensor("out_blk", [rows, COL], F32, kind="ExternalOutput").ap()

    with tile.TileContext(nc) as tc, ExitStack() as ctx:
        const = ctx.enter_context(tc.tile_pool(name="const", bufs=1))
        xnp = ctx.enter_context(tc.tile_pool(name="xn", bufs=4))
        big = ctx.enter_context(tc.tile_pool(name="big", bufs=1))
        hts = ctx.enter_context(tc.tile_pool(name="hts", bufs=1))
        onp = ctx.enter_context(tc.tile_pool(name="on", bufs=3))
        psp = ctx.enter_context(tc.tile_pool(name="ps", bufs=8, space="PSUM"))

        # constants
        ident = const.tile([128, 128], F32, name="ident", tag="ident")
        make_identity(nc, ident[:])
        wih = [const.tile([128, COL], F32, name=f"wih{c}", tag=f"wih{c}") for c in range(CT)]
        whh = [const.tile([128, COL], F32, name=f"whh{c}", tag=f"whh{c}") for c in range(CT)]
        for c in range(CT):
            nc.sync.dma_start(wih[c][:], wih_in[128 * c:128 * (c + 1), :])
            nc.sync.dma_start(whh[c][:], whh_in[128 * c:128 * (c + 1), :])
        # float32r copies (PE requires producers to round operands to f32r)
        wihr = [const.tile([128, COL], F32R, name=f"wihr{c}", tag=f"wihr{c}") for c in range(CT)]
        whhr = [const.tile([128, COL], F32R, name=f"whhr{c}", tag=f"whhr{c}") for c in range(CT)]
        for c in range(CT):
            nc.vector.tensor_copy(wihr[c][:], wih[c][:])
            nc.vector.tensor_copy(whhr[c][:], whh[c][:])
        bias = const.tile([128, CT], F32, name="bias", tag="bias")
        nc.sync.dma_start(bias[:], bias_in[:])
        amask = const.tile([128, K], F32, name="amask", tag="amask")
        nc.sync.dma_start(amask[:], amask_in[:])

        rep_ctx = tc.For_i(0, reps, 1) if reps > 1 else None
        if rep_ctx is not None:
            rep_ctx.__enter__()

        # --- phase A: load x, build X^T (f32r) via PE transposes ---
        xt = [big.tile([128, xpad], F32R, name=f"xt{c}", tag=f"xt{c}") for c in range(CT)]
        for rt in range(xpad // 128):
            xn = xnp.tile([128, COL], F32, name="xn", tag="xn")
            nc.sync.dma_start(xn[:], x_in[128 * rt:128 * (rt + 1), :])
            for c in range(CT):
                ps = psp.tile([128, 128], F32, name="ps", tag="ps", bufs=4)
                nc.tensor.transpose(ps[:], xn[:, 128 * c:128 * (c + 1)], ident[:])
                nc.vector.tensor_copy(xt[c][:, 128 * rt:128 * (rt + 1)], ps[:])

        # --- phase B: A^T = W_ih @ X^T + bias ---
        # at_all[:, o*xrows + n] = A^T[128o:128(o+1), n];  n = b*L + s
        at_all = big.tile([128, CT * xrows], F32, name="at_all", tag="at_all")
        nchunks = [(n0, min(512, xrows - n0)) for n0 in range(0, xrows, 512)]
        for o in range(CT):
            for (n0, nl) in nchunks:
                ps = psp.tile([128, 512], F32, name="ps", tag="ps", bufs=4)
                for c in range(CT):
                    nc.tensor.matmul(ps[:, :nl], wihr[c][:, 128 * o:128 * (o + 1)],
                                     xt[c][:, n0:n0 + nl],
                                     start=(c == 0), stop=(c == CT - 1))
                nc.scalar.activation(at_all[:, o * xrows + n0:o * xrows + n0 + nl],
                                     ps[:, :nl],
                                     mybir.ActivationFunctionType.Identity,
                                     bias=bias[:, o:o + 1])
        # zero the A warmup region on core 0 (amask = 0 there, 1 elsewhere)
        for o in range(CT):
            nc.vector.tensor_mul(at_all[:, o * xrows:o * xrows + K],
                                 at_all[:, o * xrows:o * xrows + K], amask[:])
        at3 = at_all.rearrange("p (o n) -> p o n", o=CT)

        # --- phase C: the lockstep recurrence ---
        ht = [[hts.tile([128, B], F32R, name=f"ht{p}{c}", tag=f"ht{p}{c}")
               for c in range(CT)] for p in range(2)]
        hz = hts.tile([128, B], F32, name="hz", tag="hz")
        nc.vector.memset(hz[:], 0.0)
        for c in range(CT):
            nc.vector.tensor_copy(ht[0][c][:], hz[:])
        htout = [big.tile([128, rows], F32, name=f"ho{o}", tag=f"ho{o}")
                 for o in range(CT)]

        for s in range(steps):
            cur, nxt = s % 2, (s + 1) % 2
            for o in range(CT):
                ps = psp.tile([128, B], F32, name="psc", tag="psc", bufs=4)
                for c in range(CT):
                    nc.tensor.matmul(ps[:], whhr[c][:, 128 * o:128 * (o + 1)],
                                     ht[cur][c][:],
                                     start=(c == 0), stop=(c == CT - 1))
                a_sl = at3[:, o, s:s + (B - 1) * L + 1:L]
                nc.vector.tensor_add(ht[nxt][o][:], ps[:], a_sl)
                nc.scalar.activation(ht[nxt][o][:], ht[nxt][o][:],
                                     mybir.ActivationFunctionType.Tanh)
                if s >= K:
                    m = s - K
                    nc.vector.tensor_copy(htout[o][:, m:m + (B - 1) * L + 1:L],
                                          ht[nxt][o][:])

        # --- phase D: transpose back to natural rows, DMA out ---
        for rt in range(rows // 128):
            on = onp.tile([128, COL], F32, name="on", tag="on")
            for o in range(CT):
                ps = psp.tile([128, 128], F32, name="ps", tag="ps", bufs=4)
                nc.tensor.transpose(ps[:], htout[o][:, 128 * rt:128 * (rt + 1)],
                                    ident[:])
                nc.vector.tensor_copy(on[:, 128 * o:128 * (o + 1)], ps[:])
            nc.sync.dma_start(out_dram[128 * rt:128 * (rt + 1), :], on[:])

        if rep_ctx is not None:
            rep_ctx.__exit__(None, None, None)

    nc.compile()
    meta = dict(B=B, K=K, L=rows // B, steps=rows // B + K, rows=rows,
                xrows=xrows, xpad=xpad)
    return nc, meta


def _host_inputs(x, W_ih, W_hh, b_ih, b_hh, meta):
    rows, xrows, xpad, Kw = meta["rows"], meta["xrows"], meta["xpad"], meta["K"]
    x = np.ascontiguousarray(x, dtype=np.float32)
    w_ihT = np.ascontiguousarray(np.asarray(W_ih, np.float32).T)
    w_hhT = np.ascontiguousarray(np.asarray(W_hh, np.float32).T)
    bv = np.asarray(b_ih, np.float32) + np.asarray(b_hh, np.float32)
    bias = np.ascontiguousarray(bv.reshape(CT, 128).T, dtype=np.float32)
    in_maps = []
    for k in range(NCORES):
        xb = np.zeros((xpad, COL), np.float32)
        lo = k * rows - Kw
        off = max(0, -lo)
        xb[off:xrows] = x[lo + off:lo + xrows]
        amask = (np.zeros if k == 0 else np.ones)((128, Kw), np.float32)
        in_maps.append({"x_blk": xb, "w_ihT": w_ihT, "w_hhT": w_hhT,
                        "bias": bias, "amask": amask})
    return in_maps


_CACHE = {}


def kernel(x, W_ih, W_hh, b_ih, b_hh):
    if "nc" not in _CACHE:
        _CACHE["nc"], _CACHE["meta"] = _build()
    nc, meta = _CACHE["nc"], _CACHE["meta"]
    in_maps = _host_inputs(x, W_ih, W_hh, b_ih, b_hh, meta)
    res = run_bass_kernel_spmd(nc, in_maps, list(range(NCORES)))
    out = np.concatenate([res.results[k]["out_blk"] for k in range(NCORES)], axis=0)
    return out.astype(np.float32)
